# revision 38
# baseline (speedup 1.0000x reference)
"""Trainium2 Bass kernel for nn_C4ByteTransformer (4-step carry-propagation
softmax table lookup).

Contract: kernel(**inputs) takes FULL inputs (a_emb[4,256], b_emb[4,256],
W1[514,131072], W2_sum[131072,256], W2_carry[131072,2]) and returns the full
[4,256] float32 output.

Math: the tables are the canonical one-hot construction (verified exactly on
host, numpy fallback otherwise), so
  scores[k] = a_i[a(k)] + b_i[b(k)] + carry[c(k)],  k = 512a + 2b + c.
The softmax weights therefore factorize rank-1 per carry slice:
  w[a,b,c] ~ alpha[a] * beta[b] * gamma_c,
  alpha = exp(10*a_i), beta = exp(10*b_i - 25), gamma_c = exp(10*carry_c),
and the entire step reduces to a length-256 circular convolution
  T[m]   = sum_a alpha[a] * beta[(m-a) & 255]          (result numerator)
plus two triangular sums for the carry:
  U1     = sum_{a+b>=256} alpha[a]*beta[b]
  U1'    = sum_{a+b>=255} alpha[a]*beta[b]
  Z      = sum(alpha) * sum(beta)
With p = P(carry=1), lam = sigmoid(10*(2p-1)) = 1/(1+exp(10-20p)):
  out_s  = (T + lam*(rot1(T) - T)) / Z
  p_next = (U1 + lam*(U1' - U1)) / Z
No table reads, no collectives: each core runs the identical tiny program
(~1.5MB of host-gathered circulant/mask operands) and core 0's output is
returned.  The convolution is one 8-chunk float32r matmul against a
host-gathered circulant of b_emb; the triangular sums are one 2-chunk matmul
against constant 0/1 masks; the 4-step carry chain runs on [1,1] scalars
with Exp only (no second ACT table load), and the cross-partition scalar
shuffles go through tiny PE-transpose matmuls (no DMA latency).
"""

import os

import numpy as np

N_CORES = 8
D = 256
NSTEP = 4
NE = 256 * 256 * 2
SCALE = 10.0
BIAS_B = -25.0

_CACHE = {}

LAST_EXEC_TIME_NS = None


def _build_nc():
    import concourse.bacc as bacc
    import concourse.mybir as mybir
    import concourse.tile as tile

    f32 = mybir.dt.float32
    f32r = mybir.dt.float32r
    f16 = mybir.dt.float16
    bf16 = mybir.dt.bfloat16
    mult = mybir.AluOpType.mult
    add = mybir.AluOpType.add
    subtract = mybir.AluOpType.subtract
    divide = mybir.AluOpType.divide
    Exp = mybir.ActivationFunctionType.Exp
    X = mybir.AxisListType.X

    nc = bacc.Bacc("TRN2", target_bir_lowering=False, debug=False,
                   num_devices=N_CORES)

    # Inputs (host pre-gathered; identical on every core).
    cb = nc.dram_tensor("cb", [128, 8, D], f16, kind="ExternalInput")
    m12 = nc.dram_tensor("m12", [128, 2, 2 * D], mybir.dt.float8e4,
                         kind="ExternalInput")
    sm128 = nc.dram_tensor("sm128", [128, 10, NSTEP], f32, kind="ExternalInput")
    bpap = nc.dram_tensor("bpap", [NSTEP, 516], f32, kind="ExternalInput")
    out = nc.dram_tensor("out", [NSTEP, D], f32, kind="ExternalOutput")

    with tile.TileContext(nc) as tc:
        with (
            tc.tile_pool(name="big", bufs=1) as big,
            tc.tile_pool(name="small", bufs=1) as small,
            tc.tile_pool(name="ps", bufs=1, space="PSUM") as ps,
        ):
            # ---- DMAs: two merged small loads first (one completion each),
            # then the big operands split across the two HWDGE queues ----
            bpap_sb = small.tile([NSTEP, 516], f32)
            nc.sync.dma_start(bpap_sb[:], bpap[:])
            sm128_sb = small.tile([128, 10, NSTEP], f32)
            nc.sync.dma_start(sm128_sb[:], sm128[:])
            m12_sb = big.tile([128, 2, 2 * D], mybir.dt.float8e4)
            nc.scalar.dma_start(m12_sb[:], m12[:])
            cb_sb = big.tile([128, 8, D], f16)
            nc.sync.dma_start(cb_sb[:, 0:4, :], cb[:, 0:4, :])
            nc.scalar.dma_start(cb_sb[:, 4:8, :], cb[:, 4:8, :])
            eye_sb = bpap_sb[:, 512:516]

            # Constant bias tiles for ACT (float biases need const APs).
            cst128 = small.tile([128, 2], f32)
            nc.vector.memset(cst128[:, 0:1], 0.0)
            nc.vector.memset(cst128[:, 1:2], BIAS_B)
            cst4 = small.tile([NSTEP, 2], f32)
            nc.vector.memset(cst4[:, 0:1], 0.0)
            nc.vector.memset(cst4[:, 1:2], BIAS_B)
            cst1 = small.tile([1, 1], f32)
            nc.vector.memset(cst1[:], 10.0)

            # ---- Exponentials (mask-matmul operands first) ----
            arm_e = small.tile([128, 2, NSTEP], bf16)
            nc.scalar.activation(arm_e[:], sm128_sb[:, 8:10, :], Exp,
                                 bias=cst128[:, 0:1], scale=SCALE)
            # Block-diagonal conv lhsT (host fills off-diag with -200 -> 0)
            at_t = small.tile([128, 8, NSTEP], f32r)
            nc.scalar.activation(at_t[:], sm128_sb[:, 0:8, :], Exp,
                                 bias=cst128[:, 0:1], scale=SCALE)
            sums = small.tile([NSTEP, 2], f32)  # (sum alpha, sum beta)
            bpe = small.tile([NSTEP, D], f32)
            nc.scalar.activation(bpe[:], bpap_sb[:, 0:D], Exp,
                                 bias=cst4[:, 1:2], scale=SCALE,
                                 accum_out=sums[:, 1:2])
            ape = small.tile([NSTEP, D], f32)
            nc.scalar.activation(ape[:], bpap_sb[:, D : 2 * D], Exp,
                                 bias=cst4[:, 0:1], scale=SCALE,
                                 accum_out=sums[:, 0:1])
            cbe = big.tile([128, 8, D], f32r)
            nc.scalar.activation(cbe[:, 0:4, :], cb_sb[:, 0:4, :], Exp,
                                 bias=cst128[:, 1:2], scale=SCALE)
            nc.scalar.activation(cbe[:, 4:8, :], cb_sb[:, 4:8, :], Exp,
                                 bias=cst128[:, 1:2], scale=SCALE)

            # ---- Matmuls: masks first (they head the long scalar-chain
            # dependency path) ----
            pm = ps.tile([NSTEP, 2, D], f32, tag="pm")
            for h in range(2):
                nc.tensor.matmul(pm[:], lhsT=arm_e[:, h, :],
                                 rhs=m12_sb[:, h, :],
                                 start=(h == 0), stop=(h == 1))

            # ---- Z, U1, U1' -> q1, q2 ----
            zz = small.tile([NSTEP, 2], f32)  # (Z, 1/Z)
            nc.vector.tensor_tensor(out=zz[:, 0:1], in0=sums[:, 0:1],
                                    in1=sums[:, 1:2], op=mult)
            nc.vector.reciprocal(zz[:, 1:2], zz[:, 0:1])
            scr = small.tile([NSTEP, 2, D], f32)
            nc.vector.tensor_tensor(out=scr[:, 0, :], in0=pm[:, 0, :],
                                    in1=bpe[:], op=mult)
            nc.vector.tensor_tensor(out=scr[:, 1, :], in0=pm[:, 1, :],
                                    in1=bpe[:], op=mult)
            uu = small.tile([NSTEP, 2], f32)  # (U1, U1')
            nc.vector.tensor_reduce(out=uu[:], in_=scr[:], axis=X,
                                    op=mybir.AluOpType.add)
            # chn = (q1, q2) = (U1/Z, (U1'-U1)/Z), each in one 2-scalar op
            chn = small.tile([NSTEP, 2], f32)
            nc.vector.tensor_scalar(out=chn[:, 0:1], in0=uu[:, 0:1],
                                    scalar1=zz[:, 1:2], scalar2=None, op0=mult)
            nc.vector.tensor_scalar(out=chn[:, 1:2], in0=uu[:, 1:2],
                                    scalar1=uu[:, 0:1], op0=subtract,
                                    scalar2=zz[:, 1:2], op1=mult)

            # ---- Repartition q1,q2 to partition-0 rows via PE transpose ----
            pt1 = ps.tile([1, NSTEP], f32, tag="pt1")
            nc.tensor.matmul(pt1[:], lhsT=chn[:, 0:1], rhs=eye_sb,
                             start=True, stop=True)
            pt2 = ps.tile([1, NSTEP], f32, tag="pt2")
            nc.tensor.matmul(pt2[:], lhsT=chn[:, 1:2], rhs=eye_sb,
                             start=True, stop=True)
            ptc1 = small.tile([1, NSTEP], f32)
            nc.vector.tensor_copy(out=ptc1[:], in_=pt1[:])
            ptc2 = small.tile([1, NSTEP], f32)
            nc.vector.tensor_copy(out=ptc2[:], in_=pt2[:])
            pc = ps.tile([NSTEP, D], f32, tag="pc")
            for c in range(8):
                nc.tensor.matmul(pc[:], lhsT=at_t[:, c, :], rhs=cbe[:, c, :],
                                 start=(c == 0), stop=(c == 7))

            # ---- Sequential carry chain on [1,1] scalars.
            # e_s = exp(10-20p); p' = q1 + q2/(1+e_s).  Exp only: no second
            # ACT table load.
            pcur = small.tile([1, 1], f32)
            nc.vector.memset(pcur[:], 0.0)
            erow = small.tile([1, NSTEP], f32)
            wtmp = small.tile([1, 2], f32)
            for s in range(NSTEP):
                nc.scalar.activation(erow[:, s : s + 1], pcur[:], Exp,
                                     bias=cst1[:], scale=-20.0)
                if s + 1 < NSTEP:
                    nc.vector.tensor_scalar(out=wtmp[:, 0:1],
                                            in0=erow[:, s : s + 1],
                                            scalar1=1.0, scalar2=None, op0=add)
                    nc.vector.reciprocal(wtmp[:, 1:2], wtmp[:, 0:1])
                    nc.vector.scalar_tensor_tensor(
                        out=pcur[:], in0=ptc2[:, s : s + 1],
                        scalar=wtmp[:, 1:2], in1=ptc1[:, s : s + 1],
                        op0=mult, op1=add,
                    )

            # ---- e back to a [4,1] column via PE; lam = 1/(1+e) ----
            pec = ps.tile([NSTEP, 1], f32, tag="pec")
            nc.tensor.matmul(pec[:], lhsT=erow[:], rhs=eye_sb[0:1, 0:1],
                             start=True, stop=True)
            ecol = small.tile([NSTEP, 2], f32)
            nc.vector.tensor_copy(out=ecol[:, 0:1], in_=pec[:])
            lamc = small.tile([NSTEP, 2], f32)
            nc.vector.tensor_scalar(out=lamc[:, 0:1], in0=ecol[:, 0:1],
                                    scalar1=1.0, scalar2=None, op0=add)
            nc.vector.reciprocal(lamc[:, 1:2], lamc[:, 0:1])

            # ---- Blend precompute (overlaps the chain): T/Z and dT/Z ----
            tsb = small.tile([NSTEP, D], f32)
            nc.vector.tensor_copy(out=tsb[:], in_=pc[:])
            trot = small.tile([NSTEP, D], f32)
            nc.vector.tensor_copy(out=trot[:, 1:D], in_=tsb[:, 0 : D - 1])
            nc.vector.tensor_copy(out=trot[:, 0:1], in_=tsb[:, D - 1 : D])
            nc.vector.tensor_tensor(out=trot[:], in0=trot[:], in1=tsb[:],
                                    op=subtract)
            res0 = small.tile([NSTEP, D], f32)
            nc.vector.tensor_scalar(out=res0[:], in0=tsb[:],
                                    scalar1=zz[:, 1:2], scalar2=None, op0=mult)
            res1 = small.tile([NSTEP, D], f32)
            nc.vector.tensor_scalar(out=res1[:], in0=trot[:],
                                    scalar1=zz[:, 1:2], scalar2=None, op0=mult)
            # ---- Final: out = res0 + lam * res1 ----
            res = small.tile([NSTEP, D], f32)
            nc.vector.scalar_tensor_tensor(out=res[:], in0=res1[:],
                                           scalar=lamc[:, 1:2], in1=res0[:],
                                           op0=mult, op1=add)
            nc.sync.dma_start(out[:], res[:])

    nc.compile()
    return nc


def _structure_ok(W1, W2_sum, W2_carry):
    """Exact check that the tables are the canonical one-hot construction."""
    if W1.shape != (514, NE) or W2_sum.shape != (NE, D) or W2_carry.shape != (NE, 2):
        return False
    k = np.arange(NE)
    a = k >> 9
    b = (k >> 1) & 255
    c = k & 1
    tot = a + b + c
    if not (W1[a, k] == 1.0).all():
        return False
    if not (W1[D + b, k] == 1.0).all():
        return False
    if not (W1[2 * D + c, k] == 1.0).all():
        return False
    if W1.sum(dtype=np.float64) != 3.0 * NE or W1.min() < 0.0:
        return False
    if not (W2_sum[k, tot & 255] == 1.0).all():
        return False
    if W2_sum.sum(dtype=np.float64) != NE or W2_sum.min() < 0.0:
        return False
    if not (W2_carry[k, (tot >= 256).astype(np.int64)] == 1.0).all():
        return False
    if W2_carry.sum(dtype=np.float64) != NE or W2_carry.min() < 0.0:
        return False
    return True


def _numpy_fallback(a_emb, b_emb, W1, W2_sum, W2_carry):
    carry = np.zeros(2, dtype=np.float64)
    carry[0] = 1.0
    outs = []
    W1 = W1.astype(np.float64)
    for i in range(NSTEP):
        x = np.concatenate([a_emb[i], b_emb[i], carry]).astype(np.float64)
        scores = x @ W1
        z = (scores - 2.5) * 10.0
        z -= z.max()
        w = np.exp(z)
        w /= w.sum()
        outs.append(w @ W2_sum.astype(np.float64))
        carry = w @ W2_carry.astype(np.float64)
    return np.stack(outs).astype(np.float32)


def _prep_inputs(a_emb, b_emb):
    """Pure-layout host gathers; every core gets the identical map."""
    idx = np.arange(D)
    p = np.arange(128)
    cb = np.empty((128, 8, D), np.float32)
    # sm128[p, c, j]: c<8 -> block-diag conv lhsT raw (off-diag -200 -> exp 0)
    #                 c=8,9 -> mask-matmul lhsT raw, [p, 8+h, s]
    sm128 = np.full((128, 10, NSTEP), -200.0, np.float32)
    for s in range(NSTEP):
        for h in range(2):
            aprime = 128 * h + p  # [128]
            cb[:, 2 * s + h, :] = b_emb[s][(idx[None, :] + aprime[:, None]) & 255]
            vals = a_emb[s][(256 - aprime) & 255]
            sm128[:, 2 * s + h, s] = vals
            sm128[:, 8 + h, s] = vals

    # m12[p, h, j] = M1[128h+p, j]; m12[p, h, 256+j] = M2[128h+p, j]
    aprime = idx[:, None]  # [256,1]
    M1 = ((idx[None, :] >= aprime) & (aprime >= 1)).astype(np.float32)
    M2 = ((idx[None, :] >= aprime - 1) & (aprime >= 1)).astype(np.float32)
    M2[0, :] = 0.0
    M2[0, 255] = 1.0
    m12 = np.empty((128, 2, 2 * D), np.float32)
    for h in range(2):
        m12[:, h, 0:D] = M1[128 * h : 128 * h + 128, :]
        m12[:, h, D : 2 * D] = M2[128 * h : 128 * h + 128, :]
    import ml_dtypes
    m12 = m12.astype(ml_dtypes.float8_e4m3fn)

    bpap = np.empty((NSTEP, 516), np.float32)
    bpap[:, 0:D] = b_emb
    bpap[:, D : 2 * D] = a_emb
    bpap[:, 512:516] = np.eye(NSTEP, dtype=np.float32)
    one = {
        "cb": cb.astype(np.float16),
        "m12": m12,
        "sm128": sm128,
        "bpap": bpap,
    }
    return [dict(one) for _ in range(N_CORES)]


def kernel(a_emb, b_emb, W1, W2_sum, W2_carry):
    global LAST_EXEC_TIME_NS
    a_emb = np.asarray(a_emb, dtype=np.float32)
    b_emb = np.asarray(b_emb, dtype=np.float32)
    W1 = np.asarray(W1, dtype=np.float32)
    W2_sum = np.asarray(W2_sum, dtype=np.float32)
    W2_carry = np.asarray(W2_carry, dtype=np.float32)

    if not _structure_ok(W1, W2_sum, W2_carry):
        return _numpy_fallback(a_emb, b_emb, W1, W2_sum, W2_carry)

    from concourse.bass_utils import run_bass_kernel_spmd

    if "nc" not in _CACHE:
        _CACHE["nc"] = _build_nc()
    nc = _CACHE["nc"]

    in_maps = _prep_inputs(a_emb, b_emb)
    trace = os.environ.get("KERNEL_TRACE", "") == "1"
    res = run_bass_kernel_spmd(nc, in_maps, list(range(N_CORES)), trace=trace)
    LAST_EXEC_TIME_NS = res.exec_time_ns
    return np.asarray(res.results[0]["out"], dtype=np.float32)


# revision 39
# speedup vs baseline: 1.0410x; 1.0410x over previous
"""Trainium2 Bass kernel for nn_C4ByteTransformer (4-step carry-propagation
softmax table lookup).

Contract: kernel(**inputs) takes FULL inputs (a_emb[4,256], b_emb[4,256],
W1[514,131072], W2_sum[131072,256], W2_carry[131072,2]) and returns the full
[4,256] float32 output.

Math: the tables are the canonical one-hot construction (verified exactly on
host, numpy fallback otherwise), so
  scores[k] = a_i[a(k)] + b_i[b(k)] + carry[c(k)],  k = 512a + 2b + c.
The softmax weights therefore factorize rank-1 per carry slice:
  w[a,b,c] ~ alpha[a] * beta[b] * gamma_c,
  alpha = exp(10*a_i), beta = exp(10*b_i - 25), gamma_c = exp(10*carry_c),
and the entire step reduces to a length-256 circular convolution
  T[m]   = sum_a alpha[a] * beta[(m-a) & 255]          (result numerator)
plus two triangular sums for the carry:
  U1     = sum_{a+b>=256} alpha[a]*beta[b]
  U1'    = sum_{a+b>=255} alpha[a]*beta[b]
  Z      = sum(alpha) * sum(beta)
With p = P(carry=1), lam = sigmoid(10*(2p-1)) = 1/(1+exp(10-20p)):
  out_s  = (T + lam*(rot1(T) - T)) / Z
  p_next = (U1 + lam*(U1' - U1)) / Z
No table reads, no collectives: each core runs the identical tiny program
(~1.5MB of host-gathered circulant/mask operands) and core 0's output is
returned.  The convolution is one 8-chunk float32r matmul against a
host-gathered circulant of b_emb; the triangular sums are one 2-chunk matmul
against constant 0/1 masks; the 4-step carry chain runs on [1,1] scalars
with Exp only (no second ACT table load), and the cross-partition scalar
shuffles go through tiny PE-transpose matmuls (no DMA latency).
"""

import os

import numpy as np

N_CORES = 8
D = 256
NSTEP = 4
NE = 256 * 256 * 2
SCALE = 10.0
BIAS_B = -25.0

_CACHE = {}

LAST_EXEC_TIME_NS = None


def _build_nc():
    import concourse.bacc as bacc
    import concourse.mybir as mybir
    import concourse.tile as tile

    f32 = mybir.dt.float32
    f32r = mybir.dt.float32r
    f16 = mybir.dt.float16
    bf16 = mybir.dt.bfloat16
    mult = mybir.AluOpType.mult
    add = mybir.AluOpType.add
    subtract = mybir.AluOpType.subtract
    divide = mybir.AluOpType.divide
    Exp = mybir.ActivationFunctionType.Exp
    X = mybir.AxisListType.X

    nc = bacc.Bacc("TRN2", target_bir_lowering=False, debug=False,
                   num_devices=N_CORES)

    # Inputs (host pre-gathered; identical on every core).
    cb = nc.dram_tensor("cb", [128, 8, D], f16, kind="ExternalInput")
    m12 = nc.dram_tensor("m12", [128, 2, 2 * D], mybir.dt.float8e4,
                         kind="ExternalInput")
    sm128 = nc.dram_tensor("sm128", [128, 10, NSTEP], f32, kind="ExternalInput")
    bpap = nc.dram_tensor("bpap", [NSTEP, 516], f32, kind="ExternalInput")
    out = nc.dram_tensor("out", [NSTEP, D], f32, kind="ExternalOutput")

    with tile.TileContext(nc) as tc:
        with (
            tc.tile_pool(name="big", bufs=1) as big,
            tc.tile_pool(name="small", bufs=1) as small,
            tc.tile_pool(name="ps", bufs=1, space="PSUM") as ps,
        ):
            # ---- DMAs: two merged small loads first (one completion each),
            # then the big operands split across the two HWDGE queues ----
            bpap_sb = small.tile([NSTEP, 516], f32)
            nc.sync.dma_start(bpap_sb[:], bpap[:])
            sm128_sb = small.tile([128, 10, NSTEP], f32)
            nc.sync.dma_start(sm128_sb[:], sm128[:])
            m12_sb = big.tile([128, 2, 2 * D], mybir.dt.float8e4)
            nc.scalar.dma_start(m12_sb[:], m12[:])
            cb_sb = big.tile([128, 8, D], f16)
            nc.sync.dma_start(cb_sb[:, 0:4, :], cb[:, 0:4, :])
            nc.scalar.dma_start(cb_sb[:, 4:8, :], cb[:, 4:8, :])
            eye_sb = bpap_sb[:, 512:516]

            # Constant bias tiles for ACT (float biases need const APs).
            cst128 = small.tile([128, 2], f32)
            nc.vector.memset(cst128[:, 0:1], 0.0)
            nc.vector.memset(cst128[:, 1:2], BIAS_B)
            cst4 = small.tile([NSTEP, 2], f32)
            nc.vector.memset(cst4[:, 0:1], 0.0)
            nc.vector.memset(cst4[:, 1:2], BIAS_B)
            cst1 = small.tile([1, 1], f32)
            nc.vector.memset(cst1[:], 10.0)

            # ---- Exponentials (mask-matmul operands first) ----
            arm_e = small.tile([128, 2, NSTEP], bf16)
            nc.scalar.activation(arm_e[:], sm128_sb[:, 8:10, :], Exp,
                                 bias=cst128[:, 0:1], scale=SCALE)
            # Block-diagonal conv lhsT (host fills off-diag with -200 -> 0)
            at_t = small.tile([128, 8, NSTEP], f32r)
            nc.scalar.activation(at_t[:], sm128_sb[:, 0:8, :], Exp,
                                 bias=cst128[:, 0:1], scale=SCALE)
            sums = small.tile([NSTEP, 2], f32)  # (sum alpha, sum beta)
            bpe = small.tile([NSTEP, D], f32)
            nc.scalar.activation(bpe[:], bpap_sb[:, 0:D], Exp,
                                 bias=cst4[:, 1:2], scale=SCALE,
                                 accum_out=sums[:, 1:2])
            ape = small.tile([NSTEP, D], f32)
            nc.scalar.activation(ape[:], bpap_sb[:, D : 2 * D], Exp,
                                 bias=cst4[:, 0:1], scale=SCALE,
                                 accum_out=sums[:, 0:1])
            cbe = big.tile([128, 8, D], f32r)
            nc.scalar.activation(cbe[:, 0:4, :], cb_sb[:, 0:4, :], Exp,
                                 bias=cst128[:, 1:2], scale=SCALE)
            nc.scalar.activation(cbe[:, 4:8, :], cb_sb[:, 4:8, :], Exp,
                                 bias=cst128[:, 1:2], scale=SCALE)

            # ---- Matmuls: masks first (they head the long scalar-chain
            # dependency path) ----
            pm = ps.tile([NSTEP, 2, D], f32, tag="pm")
            for h in range(2):
                nc.tensor.matmul(pm[:], lhsT=arm_e[:, h, :],
                                 rhs=m12_sb[:, h, :],
                                 start=(h == 0), stop=(h == 1))

            # ---- Z, U1, U1' -> q1, q2 ----
            zz = small.tile([NSTEP, 2], f32)  # (Z, 1/Z)
            nc.vector.tensor_tensor(out=zz[:, 0:1], in0=sums[:, 0:1],
                                    in1=sums[:, 1:2], op=mult)
            nc.vector.reciprocal(zz[:, 1:2], zz[:, 0:1])
            scr = small.tile([NSTEP, 2, D], f32)
            nc.vector.tensor_tensor(out=scr[:, 0, :], in0=pm[:, 0, :],
                                    in1=bpe[:], op=mult)
            nc.vector.tensor_tensor(out=scr[:, 1, :], in0=pm[:, 1, :],
                                    in1=bpe[:], op=mult)
            uu = small.tile([NSTEP, 2], f32)  # (U1, U1')
            nc.vector.tensor_reduce(out=uu[:], in_=scr[:], axis=X,
                                    op=mybir.AluOpType.add)
            # chn = (q1, q2) = (U1/Z, (U1'-U1)/Z), each in one 2-scalar op
            chn = small.tile([NSTEP, 2], f32)
            nc.vector.tensor_scalar(out=chn[:, 0:1], in0=uu[:, 0:1],
                                    scalar1=zz[:, 1:2], scalar2=None, op0=mult)
            nc.vector.tensor_scalar(out=chn[:, 1:2], in0=uu[:, 1:2],
                                    scalar1=uu[:, 0:1], op0=subtract,
                                    scalar2=zz[:, 1:2], op1=mult)

            # ---- Repartition q1,q2 to partition-0 rows via PE transpose ----
            pt1 = ps.tile([1, NSTEP], f32, tag="pt1")
            nc.tensor.matmul(pt1[:], lhsT=chn[:, 0:1], rhs=eye_sb,
                             start=True, stop=True)
            pt2 = ps.tile([1, NSTEP], f32, tag="pt2")
            pt2_mm = nc.tensor.matmul(pt2[:], lhsT=chn[:, 1:2], rhs=eye_sb,
                                      start=True, stop=True)
            ptc1 = small.tile([1, NSTEP], f32)
            nc.vector.tensor_copy(out=ptc1[:], in_=pt1[:])
            ptc2 = small.tile([1, NSTEP], f32)
            nc.vector.tensor_copy(out=ptc2[:], in_=pt2[:])
            pc = ps.tile([NSTEP, D], f32, tag="pc")
            for c in range(8):
                mm = nc.tensor.matmul(pc[:], lhsT=at_t[:, c, :],
                                      rhs=cbe[:, c, :],
                                      start=(c == 0), stop=(c == 7))
                if c == 0:
                    tile.add_dep_helper(mm.ins, pt2_mm.ins, False,
                                        "q transposes run before the conv")

            # ---- Sequential carry chain on [1,1] scalars.
            # e_s = exp(10-20p); p' = q1 + q2/(1+e_s).  Exp only: no second
            # ACT table load.
            pcur = small.tile([1, 1], f32)
            nc.vector.memset(pcur[:], 0.0)
            erow = small.tile([1, NSTEP], f32)
            wtmp = small.tile([1, 2], f32)
            for s in range(NSTEP):
                nc.scalar.activation(erow[:, s : s + 1], pcur[:], Exp,
                                     bias=cst1[:], scale=-20.0)
                if s + 1 < NSTEP:
                    nc.vector.tensor_scalar(out=wtmp[:, 0:1],
                                            in0=erow[:, s : s + 1],
                                            scalar1=1.0, scalar2=None, op0=add)
                    nc.vector.reciprocal(wtmp[:, 1:2], wtmp[:, 0:1])
                    nc.vector.scalar_tensor_tensor(
                        out=pcur[:], in0=ptc2[:, s : s + 1],
                        scalar=wtmp[:, 1:2], in1=ptc1[:, s : s + 1],
                        op0=mult, op1=add,
                    )

            # ---- e back to a [4,1] column via PE; lam = 1/(1+e) ----
            pec = ps.tile([NSTEP, 1], f32, tag="pec")
            nc.tensor.matmul(pec[:], lhsT=erow[:], rhs=eye_sb[0:1, 0:1],
                             start=True, stop=True)
            ecol = small.tile([NSTEP, 2], f32)
            nc.vector.tensor_copy(out=ecol[:, 0:1], in_=pec[:])
            lamc = small.tile([NSTEP, 2], f32)
            nc.vector.tensor_scalar(out=lamc[:, 0:1], in0=ecol[:, 0:1],
                                    scalar1=1.0, scalar2=None, op0=add)
            nc.vector.reciprocal(lamc[:, 1:2], lamc[:, 0:1])

            # ---- Blend precompute (overlaps the chain): T/Z and dT/Z ----
            tsb = small.tile([NSTEP, D], f32)
            nc.vector.tensor_copy(out=tsb[:], in_=pc[:])
            trot = small.tile([NSTEP, D], f32)
            nc.vector.tensor_copy(out=trot[:, 1:D], in_=tsb[:, 0 : D - 1])
            nc.vector.tensor_copy(out=trot[:, 0:1], in_=tsb[:, D - 1 : D])
            nc.vector.tensor_tensor(out=trot[:], in0=trot[:], in1=tsb[:],
                                    op=subtract)
            res0 = small.tile([NSTEP, D], f32)
            nc.vector.tensor_scalar(out=res0[:], in0=tsb[:],
                                    scalar1=zz[:, 1:2], scalar2=None, op0=mult)
            res1 = small.tile([NSTEP, D], f32)
            nc.vector.tensor_scalar(out=res1[:], in0=trot[:],
                                    scalar1=zz[:, 1:2], scalar2=None, op0=mult)
            # ---- Final: out = res0 + lam * res1 ----
            res = small.tile([NSTEP, D], f32)
            nc.vector.scalar_tensor_tensor(out=res[:], in0=res1[:],
                                           scalar=lamc[:, 1:2], in1=res0[:],
                                           op0=mult, op1=add)
            nc.sync.dma_start(out[:], res[:])

    nc.compile()
    return nc


def _structure_ok(W1, W2_sum, W2_carry):
    """Exact check that the tables are the canonical one-hot construction."""
    if W1.shape != (514, NE) or W2_sum.shape != (NE, D) or W2_carry.shape != (NE, 2):
        return False
    k = np.arange(NE)
    a = k >> 9
    b = (k >> 1) & 255
    c = k & 1
    tot = a + b + c
    if not (W1[a, k] == 1.0).all():
        return False
    if not (W1[D + b, k] == 1.0).all():
        return False
    if not (W1[2 * D + c, k] == 1.0).all():
        return False
    if W1.sum(dtype=np.float64) != 3.0 * NE or W1.min() < 0.0:
        return False
    if not (W2_sum[k, tot & 255] == 1.0).all():
        return False
    if W2_sum.sum(dtype=np.float64) != NE or W2_sum.min() < 0.0:
        return False
    if not (W2_carry[k, (tot >= 256).astype(np.int64)] == 1.0).all():
        return False
    if W2_carry.sum(dtype=np.float64) != NE or W2_carry.min() < 0.0:
        return False
    return True


def _numpy_fallback(a_emb, b_emb, W1, W2_sum, W2_carry):
    carry = np.zeros(2, dtype=np.float64)
    carry[0] = 1.0
    outs = []
    W1 = W1.astype(np.float64)
    for i in range(NSTEP):
        x = np.concatenate([a_emb[i], b_emb[i], carry]).astype(np.float64)
        scores = x @ W1
        z = (scores - 2.5) * 10.0
        z -= z.max()
        w = np.exp(z)
        w /= w.sum()
        outs.append(w @ W2_sum.astype(np.float64))
        carry = w @ W2_carry.astype(np.float64)
    return np.stack(outs).astype(np.float32)


def _prep_inputs(a_emb, b_emb):
    """Pure-layout host gathers; every core gets the identical map."""
    idx = np.arange(D)
    p = np.arange(128)
    cb = np.empty((128, 8, D), np.float32)
    # sm128[p, c, j]: c<8 -> block-diag conv lhsT raw (off-diag -200 -> exp 0)
    #                 c=8,9 -> mask-matmul lhsT raw, [p, 8+h, s]
    sm128 = np.full((128, 10, NSTEP), -200.0, np.float32)
    for s in range(NSTEP):
        for h in range(2):
            aprime = 128 * h + p  # [128]
            cb[:, 2 * s + h, :] = b_emb[s][(idx[None, :] + aprime[:, None]) & 255]
            vals = a_emb[s][(256 - aprime) & 255]
            sm128[:, 2 * s + h, s] = vals
            sm128[:, 8 + h, s] = vals

    # m12[p, h, j] = M1[128h+p, j]; m12[p, h, 256+j] = M2[128h+p, j]
    aprime = idx[:, None]  # [256,1]
    M1 = ((idx[None, :] >= aprime) & (aprime >= 1)).astype(np.float32)
    M2 = ((idx[None, :] >= aprime - 1) & (aprime >= 1)).astype(np.float32)
    M2[0, :] = 0.0
    M2[0, 255] = 1.0
    m12 = np.empty((128, 2, 2 * D), np.float32)
    for h in range(2):
        m12[:, h, 0:D] = M1[128 * h : 128 * h + 128, :]
        m12[:, h, D : 2 * D] = M2[128 * h : 128 * h + 128, :]
    import ml_dtypes
    m12 = m12.astype(ml_dtypes.float8_e4m3fn)

    bpap = np.empty((NSTEP, 516), np.float32)
    bpap[:, 0:D] = b_emb
    bpap[:, D : 2 * D] = a_emb
    bpap[:, 512:516] = np.eye(NSTEP, dtype=np.float32)
    one = {
        "cb": cb.astype(np.float16),
        "m12": m12,
        "sm128": sm128,
        "bpap": bpap,
    }
    return [dict(one) for _ in range(N_CORES)]


def kernel(a_emb, b_emb, W1, W2_sum, W2_carry):
    global LAST_EXEC_TIME_NS
    a_emb = np.asarray(a_emb, dtype=np.float32)
    b_emb = np.asarray(b_emb, dtype=np.float32)
    W1 = np.asarray(W1, dtype=np.float32)
    W2_sum = np.asarray(W2_sum, dtype=np.float32)
    W2_carry = np.asarray(W2_carry, dtype=np.float32)

    if not _structure_ok(W1, W2_sum, W2_carry):
        return _numpy_fallback(a_emb, b_emb, W1, W2_sum, W2_carry)

    from concourse.bass_utils import run_bass_kernel_spmd

    if "nc" not in _CACHE:
        _CACHE["nc"] = _build_nc()
    nc = _CACHE["nc"]

    in_maps = _prep_inputs(a_emb, b_emb)
    trace = os.environ.get("KERNEL_TRACE", "") == "1"
    res = run_bass_kernel_spmd(nc, in_maps, list(range(N_CORES)), trace=trace)
    LAST_EXEC_TIME_NS = res.exec_time_ns
    return np.asarray(res.results[0]["out"], dtype=np.float32)
